# revision 9
# baseline (speedup 1.0000x reference)
"""DeMash kernel for Trainium2 (8 NeuronCores, Bass/Tile).

Math: Y = X @ C^H over rows n = (B,T,S) flattened, with a subcarrier
gather before and scatter after. Real arithmetic (f32r matmuls -> full
PE rate, ~1.5e-4 rel err):
    Yr = Xr @ Cr^T + Xi @ Ci^T
    Yi = Xi @ Cr^T + (-Xr) @ Ci^T
Sharding: data-parallel over batch (axis 0), 32 batches -> 256 rows per
core; C replicated. Stationary = X^T tiles, moving = C^T slabs, PSUM
accumulation over the L=1512 contraction in 12 tiles of 126. Output
columns processed in 3 chunks of 504 (one PSUM bank each); C slabs for
chunks 0+1 ship as single ~1 MB DMAs for DMA-engine efficiency.
"""

import numpy as np
import concourse.bass as bass
import concourse.mybir as mybir
from concourse import bacc
from concourse.tile import TileContext
from concourse.bass_utils import run_bass_kernel_spmd

B, T, S, SYM, FFT = 256, 4, 2, 14, 128
NSC = 108
L = SYM * NSC                   # 1512
NCORES = 8
ROWS = (B // NCORES) * T * S    # 256 rows per core
NT = ROWS // 128                # 2 row tiles of 128
KT, KP = 12, 126                # contraction tiles: 12 x 126 = 1512
NCH, NW = 3, 504                # output column chunks: 3 x 504 = 1512

F32 = mybir.dt.float32
F32R = mybir.dt.float32r

TRACE = False                   # test harness flips this for profiling
LAST_RESULTS = None             # stashed BassKernelResults for the harness

_NC = None


def _build_program():
    nc = bacc.Bacc("TRN2", target_bir_lowering=False, debug=False)
    XX = nc.dram_tensor("XX", [KT, KP, 2 * ROWS], F32R, kind="ExternalInput")
    CC = nc.dram_tensor("CC", [KT, KP, 2 * L], F32R, kind="ExternalInput")
    YY = nc.dram_tensor("YY", [ROWS, 2 * L], F32, kind="ExternalOutput")

    def ring(j):
        return nc.sync if (j % 2 == 0) else nc.scalar

    with TileContext(nc) as tc:
        with (
            tc.tile_pool(name="xp", bufs=1) as xp,
            tc.tile_pool(name="cp", bufs=1) as cp,
            tc.tile_pool(name="c2p", bufs=12) as c2p,
            tc.tile_pool(name="op", bufs=2) as op,
            tc.tile_pool(name="pp", bufs=3, space="PSUM") as pp,
        ):
            xts, xns, c01 = [], [], []
            for k in range(KT):
                xt = xp.tile([128, 2 * ROWS], F32R, tag=f"xt{k}")
                ring(k).dma_start(out=xt[:KP, :], in_=XX[k])
                xn = xp.tile([128, ROWS], F32R, tag=f"xn{k}")
                nc.vector.tensor_scalar_mul(
                    xn[:KP, :], xt[:KP, 0:ROWS], -1.0
                )
                ct = cp.tile([128, 4 * NW], F32R, tag=f"c01_{k}")
                ring(k + 1).dma_start(out=ct[:KP, :], in_=CC[k, :, 0:4 * NW])
                xts.append(xt)
                xns.append(xn)
                c01.append(ct)

            c2 = {}

            def csl(k, mc, ri):
                if mc < 2:
                    return c01[k][:KP, mc * 2 * NW + ri * NW:
                                   mc * 2 * NW + (ri + 1) * NW]
                return c2[k][:KP, ri * NW:(ri + 1) * NW]

            def group(mc, n):
                pr = pp.tile([128, NW], F32, tag="pr")
                pi = pp.tile([128, NW], F32, tag="pi")
                for k in range(KT):
                    xr = xts[k][:KP, n * 128:n * 128 + 128]
                    xi = xts[k][:KP, ROWS + n * 128:ROWS + n * 128 + 128]
                    xnn = xns[k][:KP, n * 128:n * 128 + 128]
                    nc.tensor.matmul(pr[:], xr, csl(k, mc, 0),
                                     start=(k == 0), stop=False)
                    nc.tensor.matmul(pr[:], xi, csl(k, mc, 1),
                                     start=False, stop=(k == KT - 1))
                    nc.tensor.matmul(pi[:], xi, csl(k, mc, 0),
                                     start=(k == 0), stop=False)
                    nc.tensor.matmul(pi[:], xnn, csl(k, mc, 1),
                                     start=False, stop=(k == KT - 1))
                yy = op.tile([128, 2 * NW], F32, tag="yy")
                nc.vector.tensor_copy(out=yy[:, 0:NW], in_=pr[:])
                nc.vector.tensor_copy(out=yy[:, NW:2 * NW], in_=pi[:])
                rsl = slice(n * 128, (n + 1) * 128)
                csl2 = slice(mc * 2 * NW, (mc + 1) * 2 * NW)
                ring(mc + n).dma_start(out=YY[rsl, csl2], in_=yy[:])

            group(0, 0)
            group(0, 1)
            # prefetch chunk-2 C slabs during the dense mc=1 phase
            for k in range(KT):
                ct = c2p.tile([128, 2 * NW], F32R, tag="c2")
                ring(k).dma_start(out=ct[:KP, :], in_=CC[k, :, 4 * NW:6 * NW])
                c2[k] = ct
            group(1, 0)
            group(1, 1)
            group(2, 0)
            group(2, 1)
    nc.compile()
    return nc


def _get_nc():
    global _NC
    if _NC is None:
        _NC = _build_program()
    return _NC


def kernel(x_real, x_imag, C_real, C_imag, sc_ind):
    global LAST_RESULTS
    xr = np.asarray(x_real, dtype=np.float32)
    xi = np.asarray(x_imag, dtype=np.float32)
    cr = np.asarray(C_real, dtype=np.float32)
    ci = np.asarray(C_imag, dtype=np.float32)
    sc = np.asarray(sc_ind)

    # Host prep: gather effective subcarriers, flatten, transpose.
    idx = sc.astype(np.int64)
    xgr = xr[..., idx].reshape(B * T * S, L)      # [2048, 1512]
    xgi = xi[..., idx].reshape(B * T * S, L)
    xrT = np.ascontiguousarray(xgr.T)             # [1512, 2048]
    xiT = np.ascontiguousarray(xgi.T)

    # C^T slabs: CC[k, p, (mc, ri, j)] with Cr/Ci interleaved per chunk
    crT = cr.T.reshape(KT, KP, NCH, NW)
    ciT = ci.T.reshape(KT, KP, NCH, NW)
    CC = np.empty((KT, KP, NCH, 2, NW), dtype=np.float32)
    CC[:, :, :, 0, :] = crT
    CC[:, :, :, 1, :] = ciT
    CC = np.ascontiguousarray(CC.reshape(KT, KP, 2 * L))

    in_maps = []
    for c in range(NCORES):
        cols = slice(c * ROWS, (c + 1) * ROWS)
        XXc = np.empty((KT, KP, 2 * ROWS), dtype=np.float32)
        XXc[..., 0:ROWS] = xrT[:, cols].reshape(KT, KP, ROWS)
        XXc[..., ROWS:] = xiT[:, cols].reshape(KT, KP, ROWS)
        in_maps.append({"XX": np.ascontiguousarray(XXc), "CC": CC})

    nc = _get_nc()
    res = run_bass_kernel_spmd(
        nc, in_maps, core_ids=list(range(NCORES)), trace=TRACE
    )
    LAST_RESULTS = res

    # YY[rows, (mc, ri, j)] per core -> Yr/Yi [2048, 1512]
    yy = np.concatenate([r["YY"] for r in res.results], axis=0)
    yy = yy.reshape(B * T * S, NCH, 2, NW)
    yr_full = yy[:, :, 0, :].reshape(B * T * S, L)
    yi_full = yy[:, :, 1, :].reshape(B * T * S, L)

    out = np.zeros((2, B, T, S, SYM, FFT), dtype=np.float32)
    out[0].reshape(B * T * S, SYM, FFT)[:, :, idx] = yr_full.reshape(
        B * T * S, SYM, NSC
    )
    out[1].reshape(B * T * S, SYM, FFT)[:, :, idx] = yi_full.reshape(
        B * T * S, SYM, NSC
    )
    return out
